# revision 11
# baseline (speedup 1.0000x reference)
"""Single-level 2D Haar DWT (periodization mode) on Trainium2.

Input x: (8, 512, 512, 16) fp32 NHWC. Output: (LL, LH, HL, HH), each
(8, 256, 256, 16) fp32 — +/- combinations of each 2x2 spatial block,
scaled by 0.5.

Sharding: pure data parallel — one batch sample per NeuronCore (8 cores).

All device I/O is fp16 (host casts; the x0.5 subband scale is applied
during the host-side fp16 -> fp32 upcast): 8.4 MB in + 8.4 MB out per
core.

DMA model measured on this part (SPMD, all 8 cores streaming):
  - descriptors from all queues funnel into 16 per-engine FIFOs and
    are processed in ARRIVAL order at ~25 GB/s/engine (~425 GB/s
    aggregate = this core's HBM share);
  - SWDGE (GpSimd Q0) generates descriptors in bulk (floods the
    FIFOs, wins contention); each HWDGE ring (SP Q1 / ACT Q10)
    trickles at ~28 ns/descriptor — so HWDGE throughput is
    proportional to DESCRIPTOR SIZE: 4 KB descs ~ 150 GB/s,
    8-16 KB descs ~ 285 GB/s (engine-capped ~200 contended);
  - descriptor size = min(contiguous DRAM run, per-partition SBUF
    run) — so big descriptors need full-width rows;
  - at most ~8 outstanding SWDGE dma_starts before a framework
    DRAIN stalls the GpSimd stream.

Hence the ROW-split (not W-split): path A = rows 0:256, path B =
rows 256:512, both full width. Every load moves [128, 16 KB] rows
and every store writes full 8 KB output rows.

Path A (rows 0:256 = kc 0,1) — TensorE + ScalarE + VectorE:
  - one [128, 8192] load per kc (Q0 SWDGE, 16 KB descs);
  - per kc, 4 column-quarter units: 4 matmuls of 512 (H butterfly
    via fixed +/-1 fp16 weight; PSUM p0:63 = top+bot, p64:127 =
    top-bot), ACT PSUM->SBUF copy, 2 DVE TTs (W butterfly) into
    [128, 4096] sum/diff tiles;
  - 4 stores of [64 rows, 8 KB] per kc, spread across all queues.

Path B (rows 256:512, one unit of 128 row-pairs) — H-first butterfly:
  - top/bot [128, 8192] loads ride Q1/Q10 HWDGE, issued before the
    SWDGE flood so their descriptors land early (~13 us);
  - S = top+bot, D = top-bot computed in column halves: contiguous
    [128, 4096] ops; S-right runs on the GpSimd Pool ALU (otherwise
    idle), the rest on VectorE;
  - outs: LL=S_e+S_o, LH=S_e-S_o, HL=D_e+D_o, HH=D_e-D_o — strided
    DVE TTs into [128, 4096] per-subband tiles;
  - 4 stores of [128 rows, 8 KB].

Every tile is resident in SBUF (~193 KB/partition, no reuse). The
emission order sets per-engine priorities (B loads and kc0 first;
kc1 and B-outs fill engine gaps; A kc=1 stores ride Q0's free tail).

Bacc is built with num_devices=1: no collectives needed.
"""

import sys

if "/opt/trn_rl_repo" not in sys.path:
    sys.path.insert(0, "/opt/trn_rl_repo")

import numpy as np

B, H, W, C = 8, 512, 512, 16
N_CORES = 8
HO, WO = H // 2, W // 2  # 256, 256
ROW = W * C  # 8192 elements per input row
OROW = WO * C  # 4096 elements per output row

_CACHE = {}


def _haar_weight():
    """lhsT [k, m]: matmul computes out[m, n] = sum_k w[k, m] x[k, n]."""
    w = np.zeros((128, 128), dtype=np.float16)
    for m in range(64):
        w[2 * m, m] = 1.0
        w[2 * m + 1, m] = 1.0
        w[2 * m, 64 + m] = 1.0
        w[2 * m + 1, 64 + m] = -1.0
    return w


def _build():
    import concourse.bacc as bacc
    import concourse.mybir as mybir
    import concourse.tile as tile

    fp32 = mybir.dt.float32
    fp16 = mybir.dt.float16

    nc = bacc.Bacc(
        "TRN2", target_bir_lowering=False, debug=False, num_devices=1
    )
    x = nc.dram_tensor("x", (H, ROW), fp16, kind="ExternalInput")
    wdram = nc.dram_tensor("w", (128, 128), fp16, kind="ExternalInput")
    outs = {
        name: nc.dram_tensor(name, (HO, OROW), fp16, kind="ExternalOutput")
        for name in ("LL", "LH", "HL", "HH")
    }

    xq = x.rearrange("(q t) m -> q t m", t=2)  # [pair, row-parity, cols]

    AW = 2048  # A unit width (input cols); 4 matmuls of 512
    MM_N = 512  # one fp32 matmul / PSUM bank
    HR = ROW // 2  # 4096: column half (elements)

    with tile.TileContext(nc) as tc:
        with (
            tc.tile_pool(name="main", bufs=1) as pool,
            tc.tile_pool(name="psum", bufs=2, space="PSUM") as psum,
        ):
            wt = pool.tile([128, 128], fp16, tag="wt")
            nc.sync.dma_start(wt[:], wdram[:])

            # ---- tiles ----
            top = pool.tile([128, ROW], fp16, tag="top", name="top")
            bot = pool.tile([128, ROW], fp16, tag="bot", name="bot")
            xt = {}
            for kc in range(2):
                xt[kc] = pool.tile(
                    [128, ROW], fp16, tag=f"xt{kc}", name=f"xt{kc}"
                )
            S = pool.tile([128, ROW], fp16, tag="S", name="S")
            D = pool.tile([128, ROW], fp16, tag="D", name="D")

            # ---- loads ----
            # B loads on the HWDGE rings, issued BEFORE the SWDGE flood:
            # their 16 KB descriptors enqueue first and land ~13 us.
            nc.sync.dma_start(top[:], xq[slice(128, 256), 0, :])
            nc.scalar.dma_start(bot[:], xq[slice(128, 256), 1, :])
            # A loads on Q0 SWDGE: kc0 lands ~11, kc1 ~15.5.
            for kc in range(2):
                nc.gpsimd.dma_start(
                    xt[kc][:], x[kc * 128 : (kc + 1) * 128, :]
                )

            # ---- B H-butterfly mids (column halves, contiguous) ----
            # S-right on the Pool ALU (GpSimd is otherwise idle);
            # S-left, D-left, D-right on VectorE (highest DVE priority).
            nc.gpsimd.tensor_add(S[:, HR:ROW], top[:, HR:ROW], bot[:, HR:ROW])
            nc.vector.tensor_add(S[:, 0:HR], top[:, 0:HR], bot[:, 0:HR])
            nc.vector.tensor_sub(D[:, 0:HR], top[:, 0:HR], bot[:, 0:HR])
            nc.vector.tensor_sub(D[:, HR:ROW], top[:, HR:ROW], bot[:, HR:ROW])

            # ---- A units (kc-major; 4 column quarters each) ----
            sums = {}
            diffs = {}
            for kc in range(2):
                sums[kc] = pool.tile(
                    [128, OROW], fp16, tag=f"s{kc}", name=f"s{kc}"
                )
                diffs[kc] = pool.tile(
                    [128, OROW], fp16, tag=f"d{kc}", name=f"d{kc}"
                )

            def emit_a_unit(kc, q):
                ps = psum.tile([128, AW], fp32)
                for j in range(AW // MM_N):
                    lo = j * MM_N
                    nc.tensor.matmul(
                        ps[:, lo : lo + MM_N],
                        wt[:],
                        xt[kc][:, q * AW + lo : q * AW + lo + MM_N],
                        start=True,
                        stop=True,
                    )
                sb = pool.tile([128, AW], fp16, tag=f"sb{kc}{q}")
                nc.scalar.copy(sb[:], ps[:])  # ACT: PSUM -> SBUF, fp32->fp16
                sv_in = sb[:].rearrange("p (w u c) -> p w u c", u=2, c=C)
                ev, od = sv_in[:, :, 0, :], sv_in[:, :, 1, :]
                qs = slice(q * (AW // 2), (q + 1) * (AW // 2))
                sv = sums[kc][:, qs].rearrange("p (w c) -> p w c", c=C)
                dv = diffs[kc][:, qs].rearrange("p (w c) -> p w c", c=C)
                nc.vector.tensor_add(sv, ev, od)
                nc.vector.tensor_sub(dv, ev, od)

            def emit_a_stores(kc, rings):
                rs = slice(kc * 64, (kc + 1) * 64)
                for (name, src), ring in zip(
                    (
                        ("LL", sums[kc][0:64, :]),
                        ("HL", sums[kc][64:128, :]),
                        ("LH", diffs[kc][0:64, :]),
                        ("HH", diffs[kc][64:128, :]),
                    ),
                    rings,
                ):
                    ring.dma_start(outs[name][rs, :], src)

            for q in range(4):
                emit_a_unit(0, q)
            emit_a_stores(0, (nc.sync, nc.sync, nc.scalar, nc.scalar))

            # ---- B outs (W butterfly on S/D halves) + stores ----
            bouts = {}
            for name in ("LL", "LH", "HL", "HH"):
                bouts[name] = pool.tile(
                    [128, OROW], fp16, tag=f"b{name}", name=f"b{name}"
                )

            def emit_b_out_half(name, mid, op, h):
                src = mid[:, h * HR : (h + 1) * HR].rearrange(
                    "p (w u c) -> p w u c", u=2, c=C
                )
                ev, od = src[:, :, 0, :], src[:, :, 1, :]
                dst = bouts[name][
                    :, h * (HR // 2) : (h + 1) * (HR // 2)
                ].rearrange("p (w c) -> p w c", c=C)
                if op == "add":
                    nc.vector.tensor_add(dst, ev, od)
                else:
                    nc.vector.tensor_sub(dst, ev, od)

            # left halves first (S/D-left computed early on DVE);
            # right halves of HL/HH also early (D-right on DVE);
            # LL/LH-right wait on the Pool op.
            for h in (0, 1):
                emit_b_out_half("HL", D, "add", h)
                emit_b_out_half("HH", D, "sub", h)
            emit_b_out_half("LL", S, "add", 0)
            emit_b_out_half("LH", S, "sub", 0)
            emit_b_out_half("LL", S, "add", 1)
            emit_b_out_half("LH", S, "sub", 1)

            brs = slice(128, 256)
            nc.sync.dma_start(outs["HL"][brs, :], bouts["HL"][:])
            nc.scalar.dma_start(outs["HH"][brs, :], bouts["HH"][:])
            nc.sync.dma_start(outs["LL"][brs, :], bouts["LL"][:])
            nc.scalar.dma_start(outs["LH"][brs, :], bouts["LH"][:])

            # ---- A kc=1 (tail: shortest post-land chain) ----
            for q in range(4):
                emit_a_unit(1, q)
            # kc1 stores ride Q0's free tail + one per HWDGE ring
            emit_a_stores(1, (nc.sync, nc.gpsimd, nc.scalar, nc.gpsimd))

    nc.compile()
    return nc


def _get_nc():
    if "nc" not in _CACHE:
        _CACHE["nc"] = _build()
    return _CACHE["nc"]


def _in_maps(x):
    w = _haar_weight()
    xh = np.asarray(x, dtype=np.float16)
    return [
        {"x": np.ascontiguousarray(xh[i].reshape(H, ROW)), "w": w}
        for i in range(B)
    ]


def kernel(x):
    from concourse.bass_utils import run_bass_kernel_spmd

    x = np.asarray(x, dtype=np.float32)
    assert x.shape == (B, H, W, C), x.shape

    nc = _get_nc()
    try:
        res = run_bass_kernel_spmd(nc, _in_maps(x), list(range(N_CORES)))
    except Exception:
        # transient NRT device errors have been observed right after
        # compile; one retry has always succeeded
        res = run_bass_kernel_spmd(nc, _in_maps(x), list(range(N_CORES)))

    out = []
    for name in ("LL", "LH", "HL", "HH"):
        sub = np.stack(
            [res.results[i][name].reshape(HO, WO, C) for i in range(B)],
            axis=0,
        )
        out.append(sub.astype(np.float32) * np.float32(0.5))
    return tuple(out)


# revision 12
# speedup vs baseline: 1.2944x; 1.2944x over previous
"""Single-level 2D Haar DWT (periodization mode) on Trainium2.

Input x: (8, 512, 512, 16) fp32 NHWC. Output: (LL, LH, HL, HH), each
(8, 256, 256, 16) fp32 — +/- combinations of each 2x2 spatial block,
scaled by 0.5.

Sharding: pure data parallel — one batch sample per NeuronCore (8 cores).

All device I/O is fp16 (host casts; the x0.5 subband scale is applied
during the host-side fp16 -> fp32 upcast): 8.4 MB in + 8.4 MB out per
core.

DMA model measured on this part (SPMD, all 8 cores streaming):
  - descriptors from all queues funnel into 16 per-engine FIFOs and
    are processed in ARRIVAL order at ~25 GB/s/engine (~425 GB/s
    aggregate = this core's HBM share);
  - SWDGE (GpSimd Q0) generates descriptors in bulk and wins
    contention; each HWDGE ring (SP Q1 / ACT Q10) generates at
    ~28 ns/descriptor, so a 128-descriptor transfer occupies its
    generator for ~3.6 us — NEVER put the 256 B-row weight load at
    the head of an HWDGE queue (its 128 tiny descriptors stall the
    queue's first data load by ~8 us);
  - at most ~8 outstanding SWDGE dma_starts before a framework
    DRAIN stalls the GpSimd stream;
  - HWDGE data loads issued BEFORE the SWDGE flood land ~12.7 us;
    later ones crawl (~100 GB/s) while SWDGE streams.

Work split by W-halves across two compute paths:

Path A (W cols 0:4096) — TensorE + ScalarE + VectorE, 8 units of
  128 rows x 2048 cols (512 KB):
  - TensorE: row (H) butterfly as matmul with a fixed 128x128 +/-1
    fp16 weight (PSUM rows 0..63 = top+bot, 64..127 = top-bot).
  - ScalarE (ACT): PSUM -> SBUF copy, fp32 -> fp16.
  - VectorE: column (W) butterfly into per-kc [128, 2048] sum/diff
    tiles -> 4 merged [64 row, 4 KB] stores per kc.

Path B (W cols 4096:8192) — VectorE only, 2 units of 128 row-pairs x
  4096 cols: classic 8-op butterfly (W-first, so the first two mids
  need only the `top` tile).

Schedule: all tiles resident in SBUF (~193 KB/partition). Loads:
kc0/kc1 halves are the FIRST instruction on each HWDGE ring (land
~12.7/~18.5); Q0 carries wt, B0, kc2, kc3, B1 (8 instructions
exactly). Emission order sets per-engine priorities: B0 mids first
(DVE starts ~12), A units in land order, B outs fill DVE gaps.
Stores: A on Q1/Q10 split by kc; B LH/HH ride Q0's free tail.

Bacc is built with num_devices=1: no collectives needed.
"""

import sys

if "/opt/trn_rl_repo" not in sys.path:
    sys.path.insert(0, "/opt/trn_rl_repo")

import numpy as np

B, H, W, C = 8, 512, 512, 16
N_CORES = 8
HO, WO = H // 2, W // 2  # 256, 256
ROW = W * C  # 8192 elements per input row
OROW = WO * C  # 4096 elements per output row

_CACHE = {}


def _haar_weight():
    """lhsT [k, m]: matmul computes out[m, n] = sum_k w[k, m] x[k, n]."""
    w = np.zeros((128, 128), dtype=np.float16)
    for m in range(64):
        w[2 * m, m] = 1.0
        w[2 * m + 1, m] = 1.0
        w[2 * m, 64 + m] = 1.0
        w[2 * m + 1, 64 + m] = -1.0
    return w


def _build():
    import concourse.bacc as bacc
    import concourse.mybir as mybir
    import concourse.tile as tile

    fp32 = mybir.dt.float32
    fp16 = mybir.dt.float16

    nc = bacc.Bacc(
        "TRN2", target_bir_lowering=False, debug=False, num_devices=1
    )
    x = nc.dram_tensor("x", (H, ROW), fp16, kind="ExternalInput")
    wdram = nc.dram_tensor("w", (128, 128), fp16, kind="ExternalInput")
    outs = {
        name: nc.dram_tensor(name, (HO, OROW), fp16, kind="ExternalOutput")
        for name in ("LL", "LH", "HL", "HH")
    }

    xq = x.rearrange("(q t) m -> q t m", t=2)  # [pair, row-parity, cols]

    HALF = ROW // 2  # 4096: A path covers cols 0:HALF, B path HALF:ROW
    AW = 2048  # A unit width (input cols); 4 matmuls of 512
    MM_N = 512  # one fp32 matmul / PSUM bank

    with tile.TileContext(nc) as tc:
        with (
            tc.tile_pool(name="main", bufs=1) as pool,
            tc.tile_pool(name="psum", bufs=2, space="PSUM") as psum,
        ):
            wt = pool.tile([128, 128], fp16, tag="wt")

            # ---- tiles ----
            tops = {}
            bots = {}
            for pc in range(2):
                tops[pc] = pool.tile(
                    [128, HALF], fp16, tag=f"top{pc}", name=f"top{pc}"
                )
                bots[pc] = pool.tile(
                    [128, HALF], fp16, tag=f"bot{pc}", name=f"bot{pc}"
                )
            # kc=0,1: two half tiles (one HWDGE load each, both queues'
            # FIRST instruction); kc=2,3: one [128, 4096] tile on SWDGE.
            xth = {}
            for kc in range(2):
                for g in range(2):
                    xth[(kc, g)] = pool.tile(
                        [128, AW], fp16, tag=f"xt{kc}{g}", name=f"xt{kc}{g}"
                    )
            xtf = {}
            for kc in range(2, 4):
                xtf[kc] = pool.tile(
                    [128, HALF], fp16, tag=f"xt{kc}", name=f"xt{kc}"
                )

            def a_src(kc, g):
                if kc < 2:
                    return xth[(kc, g)][:]
                return xtf[kc][:, g * AW : (g + 1) * AW]

            # ---- loads ----
            # HWDGE first-loads (issued before the SWDGE flood):
            nc.scalar.dma_start(xth[(0, 0)][:], x[0:128, 0:AW])
            nc.sync.dma_start(xth[(0, 1)][:], x[0:128, AW : 2 * AW])
            nc.scalar.dma_start(xth[(1, 0)][:], x[128:256, 0:AW])
            nc.sync.dma_start(xth[(1, 1)][:], x[128:256, AW : 2 * AW])
            # Q0 (SWDGE): wt first (tiny descriptors, bulk-generated),
            # then B0.top (feeds DVE), kc2, B0.bot, kc3, B1.
            nc.gpsimd.dma_start(wt[:], wdram[:])
            q0 = slice(0, 128)
            q1s = slice(128, 256)
            nc.gpsimd.dma_start(tops[0][:], xq[q0, 0, HALF:ROW])
            nc.gpsimd.dma_start(
                xtf[2][:], x[256:384, 0:HALF]
            )
            nc.gpsimd.dma_start(bots[0][:], xq[q0, 1, HALF:ROW])
            nc.gpsimd.dma_start(
                xtf[3][:], x[384:512, 0:HALF]
            )
            nc.gpsimd.dma_start(tops[1][:], xq[q1s, 0, HALF:ROW])
            nc.gpsimd.dma_start(bots[1][:], xq[q1s, 1, HALF:ROW])

            # ---- B0 mids: highest DVE priority ----
            mids = {}
            for pc in range(2):
                for mt in ("t1", "t2", "u1", "u2"):
                    mids[(pc, mt)] = pool.tile(
                        [128, HALF // 2],
                        fp16,
                        tag=f"m{mt}{pc}",
                        name=f"m{mt}{pc}",
                    )

            def emit_b_mids(pc):
                tv = tops[pc][:].rearrange("p (w u c) -> p w u c", u=2, c=C)
                bv = bots[pc][:].rearrange("p (w u c) -> p w u c", u=2, c=C)
                a, b = tv[:, :, 0, :], tv[:, :, 1, :]
                c_, d = bv[:, :, 0, :], bv[:, :, 1, :]
                m = lambda mt: mids[(pc, mt)][:].rearrange(
                    "p (w c) -> p w c", c=C
                )
                # top-only ops first: they unblock as soon as `top` lands
                nc.vector.tensor_add(m("t1"), a, b)
                nc.vector.tensor_sub(m("u1"), a, b)
                nc.vector.tensor_add(m("t2"), c_, d)
                nc.vector.tensor_sub(m("u2"), c_, d)

            emit_b_mids(0)

            # ---- A units in land order ----
            sums = {}
            diffs = {}
            for kc in range(4):
                sums[kc] = pool.tile(
                    [128, AW], fp16, tag=f"s{kc}", name=f"s{kc}"
                )
                diffs[kc] = pool.tile(
                    [128, AW], fp16, tag=f"d{kc}", name=f"d{kc}"
                )

            def emit_a_unit(kc, g):
                xt = a_src(kc, g)
                ps = psum.tile([128, AW], fp32)
                for j in range(AW // MM_N):
                    lo = j * MM_N
                    nc.tensor.matmul(
                        ps[:, lo : lo + MM_N],
                        wt[:],
                        xt[:, lo : lo + MM_N],
                        start=True,
                        stop=True,
                    )
                sb = pool.tile([128, AW], fp16, tag=f"sb{kc}{g}")
                nc.scalar.copy(sb[:], ps[:])  # ACT: PSUM -> SBUF, fp32->fp16
                sv_in = sb[:].rearrange("p (w u c) -> p w u c", u=2, c=C)
                ev, od = sv_in[:, :, 0, :], sv_in[:, :, 1, :]
                half = slice(g * (AW // 2), (g + 1) * (AW // 2))
                sv = sums[kc][:, half].rearrange("p (w c) -> p w c", c=C)
                dv = diffs[kc][:, half].rearrange("p (w c) -> p w c", c=C)
                nc.vector.tensor_add(sv, ev, od)
                nc.vector.tensor_sub(dv, ev, od)

            def emit_a_stores(kc, ring):
                rs = slice(kc * 64, (kc + 1) * 64)
                cols = slice(0, AW)
                ring.dma_start(outs["LL"][rs, cols], sums[kc][0:64, :])
                ring.dma_start(outs["HL"][rs, cols], sums[kc][64:128, :])
                ring.dma_start(outs["LH"][rs, cols], diffs[kc][0:64, :])
                ring.dma_start(outs["HH"][rs, cols], diffs[kc][64:128, :])

            # kc0/kc2 stores on Q1 (sync), kc1/kc3 on Q10 (scalar): each
            # HWDGE ring drains ~2.1 MB of A stores, spread in time.
            for kc in range(4):
                emit_a_unit(kc, 0)
                emit_a_unit(kc, 1)
                emit_a_stores(kc, nc.sync if kc % 2 == 0 else nc.scalar)

            # ---- B outs: fill DVE gaps ----
            def emit_b_outs(pc, rings):
                qs = slice(pc * 128, (pc + 1) * 128)
                oc = slice(HALF // 2, OROW)
                WQ = HALF // (2 * C)
                for (name, i0, i1, op), ring in zip(
                    (
                        ("LL", "t1", "t2", "add"),
                        ("HL", "t1", "t2", "sub"),
                        ("LH", "u1", "u2", "add"),
                        ("HH", "u1", "u2", "sub"),
                    ),
                    rings,
                ):
                    ot = pool.tile([128, WQ, C], fp16, tag=f"o{name}{pc}")
                    a0 = mids[(pc, i0)][:].rearrange("p (w c) -> p w c", c=C)
                    a1 = mids[(pc, i1)][:].rearrange("p (w c) -> p w c", c=C)
                    if op == "add":
                        nc.vector.tensor_add(ot[:], a0, a1)
                    else:
                        nc.vector.tensor_sub(ot[:], a0, a1)
                    ring.dma_start(
                        outs[name][qs, oc],
                        ot[:].rearrange("p w c -> p (w c)"),
                    )

            # B0: LL/HL on the HWDGE rings, LH/HH ride Q0 after its loads
            emit_b_outs(0, (nc.sync, nc.scalar, nc.gpsimd, nc.gpsimd))
            emit_b_mids(1)
            emit_b_outs(1, (nc.sync, nc.scalar, nc.scalar, nc.sync))

    nc.compile()
    return nc


def _get_nc():
    if "nc" not in _CACHE:
        _CACHE["nc"] = _build()
    return _CACHE["nc"]


def _in_maps(x):
    w = _haar_weight()
    xh = np.asarray(x, dtype=np.float16)
    return [
        {"x": np.ascontiguousarray(xh[i].reshape(H, ROW)), "w": w}
        for i in range(B)
    ]


def kernel(x):
    from concourse.bass_utils import run_bass_kernel_spmd

    x = np.asarray(x, dtype=np.float32)
    assert x.shape == (B, H, W, C), x.shape

    nc = _get_nc()
    try:
        res = run_bass_kernel_spmd(nc, _in_maps(x), list(range(N_CORES)))
    except Exception:
        # transient NRT device errors have been observed right after
        # compile; one retry has always succeeded
        res = run_bass_kernel_spmd(nc, _in_maps(x), list(range(N_CORES)))

    out = []
    for name in ("LL", "LH", "HL", "HH"):
        sub = np.stack(
            [res.results[i][name].reshape(HO, WO, C) for i in range(B)],
            axis=0,
        )
        out.append(sub.astype(np.float32) * np.float32(0.5))
    return tuple(out)
